# revision 27
# baseline (speedup 1.0000x reference)
"""Trainium2 Bass kernel for nn_MultiHeadAttention (B=2,S=2048,D=1024,H=16, RoPE+ALiBi+causal).

Strategy: head-parallel sharding across 8 NeuronCores (2 heads/core, both batches).
Host folds mask+bias into a per-head multiplicative exp(bias) tensor (bf16),
pre-transposes x, and sums the per-core partial output projections.
Matmul datapath is bf16 (fp32 PSUM accumulation); output partials are fp32.
"""
import sys, os

for _p in ("/root/.axon_site/_ro/trn_rl_repo", "/opt/trn_rl_repo"):
    if os.path.isdir(_p) and _p not in sys.path:
        sys.path.insert(0, _p)

import numpy as np
import ml_dtypes
import concourse.bass as bass
import concourse.mybir as mybir
import concourse.tile as tile
from concourse import bacc
from concourse.bass_utils import run_bass_kernel_spmd

F32 = mybir.dt.float32
BF16 = mybir.dt.bfloat16

# Route Exp/Ln/Copy/Identity to the one table set containing them all, so the
# scalar engine never reloads activation tables mid-kernel (each reload ~1.3us).
_COMBINED_SET = "natural_log_exp_and_others"
_SHARED_FNS = {
    mybir.ActivationFunctionType.Exp, mybir.ActivationFunctionType.Ln,
    mybir.ActivationFunctionType.Copy, mybir.ActivationFunctionType.Identity,
}
_orig_get_act_tables = bacc.get_activation_tables


def _patched_get_act_tables(arch):
    t = _orig_get_act_tables(arch)
    if _COMBINED_SET in t and _SHARED_FNS <= t[_COMBINED_SET]:
        t = {name: (fns if name == _COMBINED_SET else fns - _SHARED_FNS)
             for name, fns in t.items()}
    return t


bacc.get_activation_tables = _patched_get_act_tables

B, S, D, H = 2, 2048, 1024, 16
DK = D // H            # 64
NCORES = 8
HPC = H // NCORES      # 2 heads per core
FD = HPC * DK          # 128 ctx features per core
R = B * S              # 4096 token rows
RT = R // 128          # 32 r-tiles
QB = 512               # q-block size
NQB = S // QB          # 4 q-blocks per batch


def _build(causal: bool, qk_bias: bool, stage: int = 2):
    """Build + compile the per-core SPMD kernel. Returns the compiled Bacc."""
    nc = bacc.Bacc()

    xT = nc.dram_tensor("xT", (D, R), BF16, kind="ExternalInput")
    wcat = nc.dram_tensor("wcat", (D, 3 * FD), BF16, kind="ExternalInput")
    woB = nc.dram_tensor("woB", (2 * DK, D), BF16, kind="ExternalInput")
    expb = nc.dram_tensor("expb", (HPC, S, S), BF16, kind="ExternalInput")
    cosp = nc.dram_tensor("cosp", (128, RT * (DK // 2)), BF16, kind="ExternalInput")
    sinp = nc.dram_tensor("sinp", (128, RT * (DK // 2)), BF16, kind="ExternalInput")
    ident = nc.dram_tensor("ident", (128, 128), BF16, kind="ExternalInput")
    onesd = nc.dram_tensor("onesd", (128, 64), F32, kind="ExternalInput")
    onesb = nc.dram_tensor("onesb", (128, 1), BF16, kind="ExternalInput")
    if qk_bias:
        bropeq = nc.dram_tensor("bropeq", (128, R), BF16, kind="ExternalInput")
        bropek = nc.dram_tensor("bropek", (128, R), BF16, kind="ExternalInput")
    out = nc.dram_tensor("out", (R, D), F32, kind="ExternalOutput")

    with tile.TileContext(nc) as tc:
        import contextlib
        ctx = contextlib.ExitStack()
        with ctx:
            consts = ctx.enter_context(tc.tile_pool(name="consts", bufs=1))
            persist = ctx.enter_context(tc.tile_pool(name="persist", bufs=1))

            # --- constants ---
            id_sb = consts.tile([128, 128], BF16)
            nc.sync.dma_start(out=id_sb, in_=ident[:, :])
            wc_sb = [consts.tile([128, 3 * FD], BF16, tag=f"wc{ct}", name=f"wc{ct}") for ct in range(8)]
            for ct in range(8):
                nc.sync.dma_start(out=wc_sb[ct], in_=wcat[ct * 128:(ct + 1) * 128, :])
            wo_sb = consts.tile([2 * DK, D], BF16, tag="wo", name="wo")
            nc.sync.dma_start(out=wo_sb, in_=woB[:, :])
            cos_sb = consts.tile([128, RT * 32], BF16)
            sin_sb = consts.tile([128, RT * 32], BF16)
            nc.sync.dma_start(out=cos_sb, in_=cosp[:, :])
            nc.sync.dma_start(out=sin_sb, in_=sinp[:, :])
            ones_sb = consts.tile([128, DK], F32)
            nc.sync.dma_start(out=ones_sb, in_=onesd[:, :])

            # --- persistent activation tensors ---
            QT = persist.tile([128, R], BF16, tag="QT")   # rows: h*64 + d, cols: b*2048+s
            KT = persist.tile([128, R], BF16, tag="KT")
            vaug = {}
            for rt in range(RT):
                for hh in range(HPC):
                    vaug[(rt, hh)] = persist.tile([128, DK + 1], BF16, tag=f"va{rt}_{hh}", name=f"va{rt}_{hh}")

            # =================== Phase 1: QKV projection + RoPE + transposes ===================
            with tc.tile_pool(name="p1x", bufs=3) as p1x, \
                 tc.tile_pool(name="p1n", bufs=1) as p1n, \
                 tc.tile_pool(name="p1s", bufs=4) as p1s, \
                 tc.tile_pool(name="p1ps", bufs=3, space="PSUM") as p1ps, \
                 tc.tile_pool(name="p1pt", bufs=3, space="PSUM") as p1pt:

                qknat = p1n.tile([128, RT * 256], BF16)  # col = rt*256 + {0:128 Q | 128:256 K}, d-major
                qkrot = p1n.tile([128, RT * 256], BF16)

                GRP = 8  # r-tiles per rope group
                for g in range(RT // GRP):
                    for rt in range(g * GRP, (g + 1) * GRP):
                        xts = [p1x.tile([128, 128], BF16, tag=f"x{ct}", name=f"xt{rt}_{ct}") for ct in range(8)]
                        for ct in range(8):
                            nc.sync.dma_start(
                                out=xts[ct],
                                in_=xT[ct * 128:(ct + 1) * 128, rt * 128:(rt + 1) * 128])
                        pp = p1ps.tile([128, 3 * FD], F32, tag="prj")
                        for ct in range(8):
                            nc.tensor.matmul(pp, xts[ct], wc_sb[ct],
                                             start=(ct == 0), stop=(ct == 7))
                        # drain Q,K -> qknat (bf16); V -> vaug tiles (bf16)
                        nc.vector.tensor_copy(qknat[:, rt * 256: rt * 256 + 256], pp[:, 0:256])
                        for hh in range(HPC):
                            va = vaug[(rt, hh)]
                            nc.scalar.copy(va[:, 0:DK], pp[:, 2 * FD + hh * DK: 2 * FD + (hh + 1) * DK])
                            nc.sync.dma_start(out=va[:, DK:DK + 1], in_=onesb[:, 0:1])

                    # rope on group g: Q and K separately, 6 ops each
                    # AP dims over qknat: [part][rt(8) step 256][head(2) step 64][pair(32) step 2]
                    def sl(t, qk, eo, g=g):
                        a = t[:, :]
                        return bass.AP(
                            tensor=a.tensor,
                            offset=a.offset + (g * GRP * 256 + qk * 128 + eo),
                            ap=[a.ap[0], [256, GRP], [64, 2], [2, 32]],
                        )
                    def slc(t, g=g):  # cos/sin: [part][rt(8) step 32][head 0x2][pair step 1 x32]
                        a = t[:, :]
                        return bass.AP(
                            tensor=a.tensor,
                            offset=a.offset + g * GRP * 32,
                            ap=[a.ap[0], [32, GRP], [0, 2], [1, 32]],
                        )
                    for qk in range(2):
                        s1 = p1s.tile([128, GRP * 64], BF16, tag="s1")
                        s2 = p1s.tile([128, GRP * 64], BF16, tag="s2")
                        s3 = p1s.tile([128, GRP * 64], BF16, tag="s3")
                        s4 = p1s.tile([128, GRP * 64], BF16, tag="s4")
                        nc.vector.tensor_mul(s1, sl(qknat, qk, 0), slc(cos_sb))
                        nc.vector.tensor_mul(s2, sl(qknat, qk, 1), slc(sin_sb))
                        nc.vector.tensor_sub(sl(qkrot, qk, 0), s1, s2)
                        nc.vector.tensor_mul(s3, sl(qknat, qk, 0), slc(sin_sb))
                        nc.vector.tensor_mul(s4, sl(qknat, qk, 1), slc(cos_sb))
                        nc.vector.tensor_add(sl(qkrot, qk, 1), s3, s4)

                    # transposes for the PREVIOUS group (skewed so PE never stalls on rope)
                    if g > 0:
                        for rt in range((g - 1) * GRP, g * GRP):
                            for qk, dst in ((0, QT), (1, KT)):
                                pt = p1pt.tile([128, 128], BF16, tag="pt")
                                nc.tensor.transpose(pt, qkrot[:, rt * 256 + qk * 128: rt * 256 + qk * 128 + 128], id_sb)
                                nc.scalar.copy(dst[:, rt * 128:(rt + 1) * 128], pt)

                for rt in range(RT - GRP, RT):
                    for qk, dst in ((0, QT), (1, KT)):
                        pt = p1pt.tile([128, 128], BF16, tag="pt")
                        nc.tensor.transpose(pt, qkrot[:, rt * 256 + qk * 128: rt * 256 + qk * 128 + 128], id_sb)
                        nc.scalar.copy(dst[:, rt * 128:(rt + 1) * 128], pt)

                if qk_bias:
                    brq = p1n.tile([128, R], BF16, tag="brq")
                    brk = p1n.tile([128, R], BF16, tag="brk")
                    nc.sync.dma_start(out=brq, in_=bropeq[:, :])
                    nc.sync.dma_start(out=brk, in_=bropek[:, :])
                    nc.vector.tensor_add(QT, QT, brq)
                    nc.vector.tensor_add(KT, KT, brk)

            if stage == 1:
                for i in range(4):
                    nc.gpsimd.dma_start(out=out[i * 128:(i + 1) * 128, :],
                                        in_=QT[:, i * 1024:(i + 1) * 1024])
                    nc.gpsimd.dma_start(out=out[512 + i * 128: 512 + (i + 1) * 128, :],
                                        in_=KT[:, i * 1024:(i + 1) * 1024])
                nc.gpsimd.dma_start(out=out[1024:1152, 0:65], in_=vaug[(0, 0)])
                nc.gpsimd.dma_start(out=out[1152:1280, 0:65], in_=vaug[(31, 1)])
            # =================== Phase 2: attention + output projection ===================
            if stage >= 2:
              with tc.tile_pool(name="p2eb", bufs=4) as p2eb, \
                 tc.tile_pool(name="p2e", bufs=3) as p2e, \
                 tc.tile_pool(name="p2a", bufs=6) as p2a, \
                 tc.tile_pool(name="p2c", bufs=2) as p2c, \
                 tc.tile_pool(name="p2o", bufs=3) as p2o, \
                 tc.tile_pool(name="psc", bufs=2, space="PSUM") as psc, \
                 tc.tile_pool(name="psx", bufs=1, space="PSUM") as psx, \
                 tc.tile_pool(name="pm", bufs=1, space="PSUM") as pm:

                for qb in range(NQB):
                    nkt = (qb + 1) * (QB // 128) if causal else S // 128
                    csc = {}
                    for b in range(B):
                        csc[b] = p2c.tile([2 * DK, QB], BF16, tag=f"cb{b}", name=f"cb{qb}{b}")
                    for hh in range(HPC):
                        ctx_ps = {}
                        for b in range(B):
                            ctx_ps[b] = psx.tile([DK + 1, QB], F32, tag=f"ctx{b}", name=f"ctx{qb}_{hh}{b}")
                        pend = None  # (kt, q_off, q_len, at_tiles) awaiting ctx issue
                        def issue_ctx(pkt, p_off, p_len, p_at):
                            for b in range(B):
                                nc.tensor.matmul(
                                    ctx_ps[b][:, p_off:QB],
                                    vaug[(b * (S // 128) + pkt, hh)],
                                    p_at[b][:, 0:p_len],
                                    start=(pkt == 0), stop=(pkt == nkt - 1))
                        for kt in range(nkt):
                            q_off = max(0, kt * 128 - qb * QB) if causal else 0
                            q_len = QB - q_off
                            ebt = p2eb.tile([128, QB], BF16, tag="eb", name=f"eb{qb}_{hh}_{kt}")
                            nc.sync.dma_start(
                                out=ebt[:, 0:q_len],
                                in_=expb[hh, kt * 128:(kt + 1) * 128,
                                         qb * QB + q_off: (qb + 1) * QB])
                            scp = psc.tile([128, 2 * QB], F32, tag="sc", name=f"sc{qb}_{hh}_{kt}")
                            for b in range(B):
                                nc.tensor.matmul(
                                    scp[:, b * QB: b * QB + q_len],
                                    KT[hh * DK:(hh + 1) * DK, b * S + kt * 128: b * S + (kt + 1) * 128],
                                    QT[hh * DK:(hh + 1) * DK, b * S + qb * QB + q_off: b * S + (qb + 1) * QB],
                                    start=True, stop=True)
                            if pend is not None:
                                issue_ctx(*pend)
                            ex = p2e.tile([128, 2 * QB], BF16, tag="ex", name=f"ex{qb}_{hh}_{kt}")
                            nc.scalar.activation(ex[:, 0:QB + q_len], scp[:, 0:QB + q_len],
                                                 mybir.ActivationFunctionType.Exp)
                            at_tiles = []
                            for b in range(B):
                                at = p2a.tile([128, QB], BF16, tag=f"at{b}", name=f"at{qb}_{hh}_{kt}_{b}")
                                nc.vector.tensor_mul(at[:, 0:q_len], ex[:, b * QB: b * QB + q_len],
                                                     ebt[:, 0:q_len])
                                at_tiles.append(at)
                            pend = (kt, q_off, q_len, at_tiles)
                        issue_ctx(*pend)
                        # normalize this head into csc halves (frees ctx banks for next head)
                        for b in range(B):
                            cp = ctx_ps[b]
                            dsb = p2c.tile([DK + 1, QB], F32, tag=f"ds{b}", name=f"ds{qb}{hh}{b}")
                            nc.vector.tensor_copy(dsb[DK:DK + 1, :], cp[DK:DK + 1, :])
                            # transpose den row across partitions: dT[p, j] = den[p + 128 j]
                            dT = p2c.tile([128, 4], F32, tag=f"dT{b}", name=f"dT{qb}{hh}{b}")
                            a = dsb[DK:DK + 1, :]
                            for j in range(4):
                                nc.sync.dma_start(
                                    out=dT[:, j:j + 1],
                                    in_=bass.AP(tensor=a.tensor, offset=a.offset + 128 * j,
                                                ap=[a.ap[0], [1, 128]]))
                            rT = p2c.tile([128, 4], F32, tag=f"rT{b}", name=f"rT{qb}{hh}{b}")
                            nc.vector.reciprocal(rT, dT)
                            rc = p2c.tile([DK + 1, QB], F32, tag=f"rc{b}", name=f"rc{qb}{hh}{b}")
                            rca = rc[DK:DK + 1, :]
                            for j in range(4):
                                nc.sync.dma_start(
                                    out=bass.AP(tensor=rca.tensor, offset=rca.offset + 128 * j,
                                                ap=[rca.ap[0], [1, 128]]),
                                    in_=rT[:, j:j + 1])
                            rbp = pm.tile([DK, QB], F32, tag="rb")
                            nc.tensor.matmul(rbp, ones_sb[DK:DK + 1, :], rc[DK:DK + 1, :],
                                             start=True, stop=True)
                            rb = p2c.tile([DK, QB], F32, tag=f"rb{b}", name=f"rb{qb}{hh}{b}")
                            nc.vector.tensor_copy(rb, rbp)
                            if hh == 0:
                                nc.vector.tensor_mul(csc[b][0:DK, :], cp[0:DK, :], rb)
                            else:
                                cs1 = p2c.tile([DK, QB], BF16, tag=f"cs{b}", name=f"cs{qb}{hh}{b}")
                                nc.vector.tensor_mul(cs1, cp[0:DK, :], rb)
                                nc.sync.dma_start(out=csc[b][DK:2 * DK, :], in_=cs1)
                    for b in range(B):
                        for rs in range(QB // 128):
                            ot = p2o.tile([128, D], F32, tag="ot")
                            for eh in range(2):
                                op = pm.tile([128, 512], F32, tag="op", name=f"op{qb}{b}{rs}{eh}")
                                nc.tensor.matmul(op, csc[b][:, rs * 128:(rs + 1) * 128],
                                                 wo_sb[:, eh * 512:(eh + 1) * 512],
                                                 start=True, stop=True)
                                if eh == 0:
                                    nc.vector.tensor_copy(ot[:, 0:512], op)
                                else:
                                    nc.scalar.copy(ot[:, 512:1024], op)
                            nc.sync.dma_start(
                                out=out[b * S + qb * QB + rs * 128: b * S + qb * QB + (rs + 1) * 128, :],
                                in_=ot)
    nc.compile()
    return nc


def _build_fast(qk_bias: bool):
    """Causal+ALiBi specialized kernel.

    Head slots per core: hh=0 steep head (slope>=2^-4), banded to the last 8
    k-tiles; hh=1 shallow head (slope<=2^-4.5), ALiBi applied via a
    per-partition linear bias folded into the exp (the per-query factor
    cancels in softmax normalization), so its off-diagonal tiles need no
    elementwise multiply at all.
    """
    nc = bacc.Bacc()

    xp = nc.dram_tensor("xp", (128, RT * 1024), BF16, kind="ExternalInput")
    wc2 = nc.dram_tensor("wc2", (128, 8 * 384), BF16, kind="ExternalInput")
    woB = nc.dram_tensor("woB", (128, D), BF16, kind="ExternalInput")
    cosp = nc.dram_tensor("cosp", (128, RT * 64), BF16, kind="ExternalInput")
    sinp = nc.dram_tensor("sinp", (128, RT * 64), BF16, kind="ExternalInput")
    ident = nc.dram_tensor("ident", (128, 128), BF16, kind="ExternalInput")
    onesd = nc.dram_tensor("onesd", (128, 64), F32, kind="ExternalInput")
    eual = nc.dram_tensor("eual", (128, 3 * 512), BF16, kind="ExternalInput")
    biasv = nc.dram_tensor("biasv", (128, 40), F32, kind="ExternalInput")
    if qk_bias:
        bropeq = nc.dram_tensor("bropeq", (128, R), BF16, kind="ExternalInput")
        bropek = nc.dram_tensor("bropek", (128, R), BF16, kind="ExternalInput")
    out = nc.dram_tensor("out", (R, D), BF16, kind="ExternalOutput")

    with tile.TileContext(nc) as tc:
        import contextlib
        ctx = contextlib.ExitStack()
        with ctx:
            consts = ctx.enter_context(tc.tile_pool(name="consts", bufs=1))
            persist = ctx.enter_context(tc.tile_pool(name="persist", bufs=1))

            # --- constants ---
            id_sb = consts.tile([128, 128], BF16)
            nc.scalar.dma_start(out=id_sb, in_=ident[:, :])
            wc_sb = consts.tile([128, 8 * 384], BF16, tag="wc", name="wc")
            nc.scalar.dma_start(out=wc_sb, in_=wc2[:, :])
            cos_sb = consts.tile([128, RT * 64], BF16)
            sin_sb = consts.tile([128, RT * 64], BF16)
            nc.scalar.dma_start(out=cos_sb, in_=cosp[:, :])
            nc.scalar.dma_start(out=sin_sb, in_=sinp[:, :])
            ones_sb = consts.tile([128, DK], F32)
            nc.scalar.dma_start(out=ones_sb, in_=onesd[:, :])
            bv_sb = consts.tile([128, 40], F32, tag="bv", name="bv")
            nc.scalar.dma_start(out=bv_sb, in_=biasv[:, :])
            # needed only in phase 2 — loaded late so x tiles go first
            wo_sb = consts.tile([128, D], BF16, tag="wo", name="wo")
            eu_sb = consts.tile([128, 3 * 512], BF16, tag="eu", name="eu")

            # --- persistent activation tensors ---
            # QT/KT rows (after E/O-deinterleaved projection):
            #   [Qe_h0 0:32 | Qe_h1 32:64 | Qo_h0 64:96 | Qo_h1 96:128]
            QT = persist.tile([128, R], BF16, tag="QT")
            KT = persist.tile([128, R], BF16, tag="KT")
            # per-head dup tiles: rows 0:64 = [e|o] of head hh (for batch 0 / T0),
            # rows 64:128 = same (for batch 1 / T8)
            QTd = [persist.tile([128, R], BF16, tag=f"QTd{hh}", name=f"QTd{hh}") for hh in range(2)]
            KTd = [persist.tile([128, R], BF16, tag=f"KTd{hh}", name=f"KTd{hh}") for hh in range(2)]

            def dup_group(g, GRP=8):
                c0, c1 = g * GRP * 128, (g + 1) * GRP * 128
                for src, dsts in ((QT, QTd), (KT, KTd)):
                    for hh in range(2):
                        for half in range(2):
                            for eo in range(2):
                                nc.gpsimd.dma_start(
                                    out=dsts[hh][half * 64 + eo * 32: half * 64 + eo * 32 + 32, c0:c1],
                                    in_=src[eo * 64 + hh * 32: eo * 64 + hh * 32 + 32, c0:c1])
            # vaug: one tile; slice (rt, hh) at col (rt*2+hh)*65, 65 wide
            vaug = persist.tile([128, RT * 2 * 65], BF16, tag="va", name="va")
            va_ones = bass.AP(tensor=vaug[:, :].tensor,
                              offset=vaug[:, :].offset + 64,
                              ap=[vaug[:, :].ap[0], [65, RT * 2]])
            nc.vector.memset(va_ones, 1.0)

            def va_sl(rt, hh):
                return vaug[:, (rt * 2 + hh) * 65:(rt * 2 + hh) * 65 + 65]

            # ============ Phase 1: QKV projection + RoPE + transposes ============
            with tc.tile_pool(name="p1x", bufs=3) as p1x, \
                 tc.tile_pool(name="p1n", bufs=1) as p1n, \
                 tc.tile_pool(name="p1s", bufs=4) as p1s, \
                 tc.tile_pool(name="p1ps", bufs=3, space="PSUM") as p1ps, \
                 tc.tile_pool(name="p1pt", bufs=3, space="PSUM") as p1pt:

                qknat = p1n.tile([128, RT * 256], BF16)
                qkrot = p1n.tile([128, RT * 256], BF16)

                GRP = 8

                def do_transpose(rt):
                    for qk, dst in ((0, QT), (1, KT)):
                        pt = p1pt.tile([128, 128], BF16, tag="pt")
                        nc.tensor.transpose(pt, qkrot[:, rt * 256 + qk * 128: rt * 256 + qk * 128 + 128], id_sb)
                        if qk == 0:
                            nc.scalar.copy(dst[:, rt * 128:(rt + 1) * 128], pt)
                        else:
                            nc.vector.tensor_copy(dst[:, rt * 128:(rt + 1) * 128], pt)

                for g in range(RT // GRP):
                    for rt in range(g * GRP, (g + 1) * GRP):
                        xts = p1x.tile([128, 1024], BF16, tag="x", name=f"xt{rt}")
                        nc.sync.dma_start(out=xts, in_=xp[:, rt * 1024:(rt + 1) * 1024])
                        pp = p1ps.tile([128, 3 * FD], F32, tag="prj")
                        for ct in range(8):
                            nc.tensor.matmul(pp, xts[:, ct * 128:(ct + 1) * 128],
                                             wc_sb[:, ct * 384:(ct + 1) * 384],
                                             start=(ct == 0), stop=(ct == 7))
                        if g > 0:
                            do_transpose(rt - GRP)
                        nc.vector.tensor_copy(qknat[:, rt * 256: rt * 256 + 256], pp[:, 0:256])
                        # V drain: one op covers both heads (65-strided dest)
                        vd = va_sl(rt, 0)[:, 0:DK]
                        vdst = bass.AP(tensor=vd.tensor, offset=vd.offset,
                                       ap=[vd.ap[0], [65, 2], [1, DK]])
                        vsrc_a = pp[:, 2 * FD: 2 * FD + 128]
                        vsrc = bass.AP(tensor=vsrc_a.tensor, offset=vsrc_a.offset,
                                       ap=[vsrc_a.ap[0], [64, 2], [1, DK]])
                        nc.scalar.copy(vdst, vsrc)

                    def sl(t, qk, eo, g=g):
                        a = t[:, :]
                        return bass.AP(
                            tensor=a.tensor,
                            offset=a.offset + (g * GRP * 256 + qk * 128 + eo * 64),
                            ap=[a.ap[0], [256, GRP], [1, 64]],
                        )
                    def slc(t, g=g):
                        a = t[:, :]
                        return bass.AP(
                            tensor=a.tensor,
                            offset=a.offset + g * GRP * 64,
                            ap=[a.ap[0], [64, GRP], [1, 64]],
                        )
                    for qk in range(2):
                        s1 = p1s.tile([128, GRP * 64], BF16, tag="s1")
                        s2 = p1s.tile([128, GRP * 64], BF16, tag="s2")
                        s3 = p1s.tile([128, GRP * 64], BF16, tag="s3")
                        s4 = p1s.tile([128, GRP * 64], BF16, tag="s4")
                        nc.vector.tensor_mul(s1, sl(qknat, qk, 0), slc(cos_sb))
                        nc.vector.tensor_mul(s2, sl(qknat, qk, 1), slc(sin_sb))
                        nc.vector.tensor_sub(sl(qkrot, qk, 0), s1, s2)
                        nc.vector.tensor_mul(s3, sl(qknat, qk, 0), slc(sin_sb))
                        nc.vector.tensor_mul(s4, sl(qknat, qk, 1), slc(cos_sb))
                        nc.vector.tensor_add(sl(qkrot, qk, 1), s3, s4)

                    if g == 0:
                        nc.scalar.dma_start(out=wo_sb, in_=woB[:, :])
                        nc.scalar.dma_start(out=eu_sb, in_=eual[:, :])
                    if g > 0 and not qk_bias:
                        dup_group(g - 1)

                for rt in range(RT - GRP, RT):
                    do_transpose(rt)
                if not qk_bias:
                    dup_group(RT // GRP - 1)

                if qk_bias:
                    brq = p1n.tile([128, R], BF16, tag="brq")
                    brk = p1n.tile([128, R], BF16, tag="brk")
                    nc.sync.dma_start(out=brq, in_=bropeq[:, :])
                    nc.sync.dma_start(out=brk, in_=bropek[:, :])
                    nc.vector.tensor_add(QT, QT, brq)
                    nc.vector.tensor_add(KT, KT, brk)
                    for g in range(RT // GRP):
                        dup_group(g)

            # ============ Phase 2: attention + output projection ============
            with tc.tile_pool(name="p2e", bufs=3) as p2e, \
                 tc.tile_pool(name="p2a", bufs=3) as p2a, \
                 tc.tile_pool(name="p2c", bufs=2) as p2c, \
                 tc.tile_pool(name="p2o", bufs=3) as p2o, \
                 tc.tile_pool(name="psc", bufs=2, space="PSUM") as psc, \
                 tc.tile_pool(name="psx", bufs=1, space="PSUM") as psx, \
                 tc.tile_pool(name="pm", bufs=1, space="PSUM") as pm:

                def seg_ap(t, q_len, off=0):
                    a = t[:, :]
                    return bass.AP(tensor=a.tensor, offset=a.offset + off,
                                   ap=[a.ap[0], [QB, 2], [1, q_len]])

                # deferred norm/outproj steps, interleaved into later tile loops so
                # the PE stream never stalls on a normalization dependency chain
                pending = []

                def drain_steps(k):
                    for _ in range(min(k, len(pending))):
                        pending.pop(0)()

                def norm_steps(qb, hh, cp, csc_b, b):
                    st = {}
                    def s1():
                        dsb = p2c.tile([DK + 1, QB], F32, tag=f"ds{b}", name=f"ds{qb}{hh}{b}")
                        nc.vector.tensor_copy(dsb[DK:DK + 1, :], cp[DK:DK + 1, :])
                        lnr = p2c.tile([DK + 1, QB], F32, tag=f"ln{b}", name=f"ln{qb}{hh}{b}")
                        nc.scalar.activation(lnr[DK:DK + 1, :], dsb[DK:DK + 1, :],
                                             mybir.ActivationFunctionType.Ln)
                        st["ln"] = lnr
                    def s2():
                        lnr = st["ln"]
                        rbp = pm.tile([DK, QB], F32, tag="rb")
                        nc.tensor.matmul(rbp, ones_sb[DK:DK + 1, :], lnr[DK:DK + 1, :],
                                         start=True, stop=True)
                        rb = p2c.tile([DK, QB], F32, tag=f"rb{b}", name=f"rb{qb}{hh}{b}")
                        nc.scalar.activation(rb, rbp, mybir.ActivationFunctionType.Exp,
                                             scale=-1.0)
                        st["rb"] = rb
                    def s3():
                        rb = st["rb"]
                        if hh == 0:
                            nc.vector.tensor_mul(csc_b[0:DK, :], cp[0:DK, :], rb)
                        else:
                            cs1 = p2c.tile([DK, QB], BF16, tag=f"cs{b}", name=f"cs{qb}{hh}{b}")
                            nc.vector.tensor_mul(cs1, cp[0:DK, :], rb)
                            nc.sync.dma_start(out=csc_b[DK:2 * DK, :], in_=cs1)
                    return [s1, s2, s3]

                def outproj_steps(qb, csc):
                    steps = []
                    for b in range(B):
                        for rs in range(QB // 128):
                            def st(b=b, rs=rs, qb=qb, csc=csc):
                                ot = p2o.tile([128, D], BF16, tag="ot")
                                for eh in range(2):
                                    op = pm.tile([128, 512], F32, tag="op", name=f"op{qb}{b}{rs}{eh}")
                                    nc.tensor.matmul(op, csc[b][:, rs * 128:(rs + 1) * 128],
                                                     wo_sb[:, eh * 512:(eh + 1) * 512],
                                                     start=True, stop=True)
                                    if eh == 0:
                                        nc.vector.tensor_copy(ot[:, 0:512], op)
                                    else:
                                        nc.vector.tensor_copy(ot[:, 512:1024], op)
                                nc.gpsimd.dma_start(
                                    out=out[b * S + qb * QB + rs * 128: b * S + qb * QB + (rs + 1) * 128, :],
                                    in_=ot)
                            steps.append(st)
                    return steps

                csc_all = {}
                for qb in range(NQB):
                    nkt = (qb + 1) * 4
                    csc = {}
                    for b in range(B):
                        csc[b] = p2c.tile([2 * DK, QB], BF16, tag=f"cb{b}", name=f"cb{qb}{b}")
                    csc_all[qb] = csc
                    for hh in ((1, 0) if qb == NQB - 1 else (0, 1)):
                        kt_lo = max(0, nkt - 8) if hh == 0 else 0
                        ctx_pair = psx.tile([DK + 1, 2 * QB], F32, tag="ctx", name=f"ctx{qb}_{hh}")
                        ctx_ps = {b: ctx_pair[:, b * QB:(b + 1) * QB] for b in range(B)}
                        pend = None
                        def issue_ctx(pkt, p_off, p_len, p_rhs, kt_lo=kt_lo, nkt=nkt, hh=hh, ctx_ps=ctx_ps):
                            for b in range(B):
                                nc.tensor.matmul(
                                    ctx_ps[b][:, p_off:QB],
                                    va_sl(b * (S // 128) + pkt, hh),
                                    p_rhs[:, b * QB: b * QB + p_len],
                                    start=(pkt == kt_lo), stop=(pkt == nkt - 1))
                        for kt in range(kt_lo, nkt):
                            diag = kt >= 4 * qb
                            q_off = max(0, kt * 128 - qb * QB)
                            q_len = QB - q_off
                            m = 0 if diag else 4 * qb - kt
                            k_idx = m if hh == 0 else m - (q_off // 128)
                            col = hh * 20 + k_idx + 3
                            scp = psc.tile([128, 2 * QB], F32, tag="sc", name=f"sc{qb}_{hh}_{kt}")
                            for b in range(B):
                                # b=0 on PE row-group 0, b=1 on row-group 64: concurrent
                                nc.tensor.matmul(
                                    scp[:, b * QB: b * QB + q_len],
                                    KTd[hh][64 * b:64 * b + 64, b * S + kt * 128: b * S + (kt + 1) * 128],
                                    QTd[hh][64 * b:64 * b + 64, b * S + qb * QB + q_off: b * S + (qb + 1) * QB],
                                    start=True, stop=True, tile_position=(64 * b, 0))
                            if pend is not None:
                                issue_ctx(*pend)
                            drain_steps(2)
                            ex = p2e.tile([128, 2 * QB], BF16, tag="ex", name=f"ex{qb}_{hh}_{kt}")
                            nc.scalar.activation(seg_ap(ex, q_len), seg_ap(scp, q_len),
                                                 mybir.ActivationFunctionType.Exp,
                                                 bias=bv_sb[:, col:col + 1])
                            if hh == 0 or diag:
                                # eu blocks: [EU0 | EM0 | Mbin]
                                blk = (1 if diag else 0) if hh == 0 else 2
                                ea = eu_sb[:, blk * QB: blk * QB + QB]
                                eap = bass.AP(tensor=ea.tensor, offset=ea.offset,
                                              ap=[ea.ap[0], [0, 2], [1, q_len]])
                                at = p2a.tile([128, 2 * QB], BF16, tag="at", name=f"at{qb}_{hh}_{kt}")
                                nc.vector.tensor_mul(seg_ap(at, q_len), seg_ap(ex, q_len), eap)
                                rhs_t = at
                            else:
                                rhs_t = ex
                            pend = (kt, q_off, q_len, rhs_t)
                        issue_ctx(*pend)
                        for b in range(B):
                            pending.extend(norm_steps(qb, hh, ctx_ps[b], csc[b], b))
                    pending.extend(outproj_steps(qb, csc))
                while pending:
                    pending.pop(0)()
    nc.compile()
    return nc


_CACHE = {}


def _get_kernel(causal: bool, qk_bias: bool):
    key = (causal, qk_bias)
    if key not in _CACHE:
        _CACHE[key] = _build(causal, qk_bias)
    return _CACHE[key]


def _get_kernel_fast(qk_bias: bool):
    key = ("fast", qk_bias)
    if key not in _CACHE:
        _CACHE[key] = _build_fast(qk_bias)
    return _CACHE[key]


def _host_prep(x, mask, bias, rope_freqs, Wq, bq, Wk, bk, Wv, bv, Wo, bo, causal):
    """Build the 8 per-core input maps."""
    bf = ml_dtypes.bfloat16
    xf = np.ascontiguousarray(x.reshape(R, D).T.astype(bf))  # (D, R)
    cosf = np.cos(rope_freqs.astype(np.float32))  # (S, 32)
    sinf = np.sin(rope_freqs.astype(np.float32))
    rr = np.arange(R)
    cs_full = cosf[rr % S]  # (R, 32)
    sn_full = sinf[rr % S]
    cosp = np.ascontiguousarray(
        cs_full.reshape(RT, 128, 32).transpose(1, 0, 2).reshape(128, RT * 32).astype(bf))
    sinp = np.ascontiguousarray(
        sn_full.reshape(RT, 128, 32).transpose(1, 0, 2).reshape(128, RT * 32).astype(bf))
    identm = np.eye(128, dtype=np.float32).astype(bf)

    qk_bias = bool(np.any(bq) or np.any(bk))
    maskT = (mask != 0).T  # (k, q)

    in_maps = []
    for c in range(NCORES):
        h0 = c * HPC
        fsl = slice(c * FD, (c + 1) * FD)
        wq = Wq[fsl, :].astype(np.float32) / np.sqrt(np.float32(DK))
        wk = Wk[fsl, :].astype(np.float32)
        wv = Wv[fsl, :].astype(np.float32)
        wcat = np.ascontiguousarray(np.concatenate([wq, wk, wv], axis=0).T.astype(bf))  # (D, 384)
        wob = np.ascontiguousarray(Wo[:, c * FD: (c + 1) * FD].T.astype(bf))  # (128, D)
        eb = np.empty((HPC, S, S), dtype=bf)
        for j in range(HPC):
            bT = bias[h0 + j].T.astype(np.float32)  # (k, q)
            eb[j] = np.where(maskT, np.exp(np.minimum(bT, np.float32(80.0))),
                             np.float32(0)).astype(bf)
        m = {
            "xT": xf, "wcat": wcat, "woB": wob,
            "expb": eb, "cosp": cosp, "sinp": sinp, "ident": identm,
            "onesd": np.ones((128, 64), dtype=np.float32),
            "onesb": np.ones((128, 1), dtype=bf),
        }
        if qk_bias:
            for name, bvec in (("bropeq", bq / np.sqrt(np.float32(DK))), ("bropek", bk)):
                bt = np.empty((128, R), dtype=np.float32)
                bb = bvec[fsl].astype(np.float32).reshape(HPC, DK // 2, 2)
                for j in range(HPC):
                    be = bb[j, :, 0][None, :]  # (1, 32)
                    bo_ = bb[j, :, 1][None, :]
                    rot_e = be * cs_full - bo_ * sn_full   # (R, 32)
                    rot_o = be * sn_full + bo_ * cs_full
                    blk = np.empty((R, DK), dtype=np.float32)
                    blk[:, 0::2] = rot_e
                    blk[:, 1::2] = rot_o
                    bt[j * DK:(j + 1) * DK, :] = blk.T
                m[name] = np.ascontiguousarray(bt.astype(bf))
        in_maps.append(m)
    return in_maps, qk_bias


_SLOPES = 2.0 ** (-8.0 * (np.arange(1, H + 1) / H))  # float64, matches reference


def _is_alibi(bias):
    pos = np.arange(S)
    rel = (pos[None, :] - pos[:, None]).astype(np.float64)
    for h in range(H):
        if not np.allclose(bias[h], (_SLOPES[h] * rel).astype(np.float32),
                           rtol=1e-5, atol=1e-6):
            return False
    return True


def _host_prep_fast(x, rope_freqs, Wq, bq, Wk, bk, Wv, bv, Wo, bo):
    bf = ml_dtypes.bfloat16
    xp = np.ascontiguousarray(
        x.reshape(RT, 128, 8, 128).transpose(3, 0, 2, 1).reshape(128, RT * 1024)
        .astype(bf))
    cosf = np.cos(rope_freqs.astype(np.float32))
    sinf = np.sin(rope_freqs.astype(np.float32))
    rr = np.arange(R)
    cs_full = cosf[rr % S]
    sn_full = sinf[rr % S]
    # cos/sin duplicated per head: col rt*64 + j holds freq j%32 at token rt*128+p
    cs2 = np.concatenate([cs_full, cs_full], axis=1)  # (R, 64)
    sn2 = np.concatenate([sn_full, sn_full], axis=1)
    cosp = np.ascontiguousarray(
        cs2.reshape(RT, 128, 64).transpose(1, 0, 2).reshape(128, RT * 64).astype(bf))
    sinp = np.ascontiguousarray(
        sn2.reshape(RT, 128, 64).transpose(1, 0, 2).reshape(128, RT * 64).astype(bf))
    identm = np.eye(128, dtype=np.float32).astype(bf)
    qk_bias = bool(np.any(bq) or np.any(bk))

    pcol = np.arange(128, dtype=np.float64)[:, None]
    fcol = np.arange(QB, dtype=np.float64)[None, :]

    in_maps = []
    for c in range(NCORES):
        h0, h1 = c, 8 + c
        rows = np.r_[64 * h0:64 * h0 + 64, 64 * h1:64 * h1 + 64]
        # E/O-deinterleaved rows for Q,K: [h0-even, h1-even, h0-odd, h1-odd]
        ev, od = np.arange(0, 64, 2), np.arange(1, 64, 2)
        rows_eo = np.r_[64 * h0 + ev, 64 * h1 + ev, 64 * h0 + od, 64 * h1 + od]
        s0, s1 = _SLOPES[h0], _SLOPES[h1]
        wq = Wq[rows_eo, :].astype(np.float32) / np.sqrt(np.float32(DK))
        wk = Wk[rows_eo, :].astype(np.float32)
        wv = Wv[rows, :].astype(np.float32)
        wcat = np.concatenate([wq, wk, wv], axis=0).T  # (1024, 384)
        wc2 = np.ascontiguousarray(
            wcat.reshape(8, 128, 384).transpose(1, 0, 2).reshape(128, 8 * 384).astype(bf))
        wob = np.ascontiguousarray(Wo[:, rows].T.astype(bf))  # (128, 1024)
        eu0 = np.exp(s0 * (pcol - fcol - 64.0))
        em0 = np.where(pcol <= fcol, eu0, 0.0)
        mbin = (pcol <= fcol).astype(np.float64)
        eual = np.ascontiguousarray(
            np.concatenate([eu0, em0, mbin], axis=1).astype(np.float32).astype(bf))
        bv40 = np.zeros((128, 40), dtype=np.float32)
        for k in range(-3, 17):
            bv40[:, k + 3] = np.float32(s0 * (64.0 - 128.0 * k))
            bv40[:, 20 + k + 3] = (s1 * (pcol[:, 0] - 63.0 - 128.0 * k)).astype(np.float32)
        m = {
            "xp": xp, "wc2": wc2, "woB": wob,
            "cosp": cosp, "sinp": sinp, "ident": identm,
            "onesd": np.ones((128, 64), dtype=np.float32),
            "eual": eual, "biasv": bv40,
        }
        if qk_bias:
            for name, bvec in (("bropeq", bq / np.sqrt(np.float32(DK))), ("bropek", bk)):
                bt = np.empty((128, R), dtype=np.float32)
                bb = bvec[rows].astype(np.float32).reshape(HPC, DK // 2, 2)
                for j in range(HPC):
                    be = bb[j, :, 0][None, :]
                    bo_ = bb[j, :, 1][None, :]
                    rot_e = be * cs_full - bo_ * sn_full  # (R, 32)
                    rot_o = be * sn_full + bo_ * cs_full
                    bt[32 * j:32 * j + 32, :] = rot_e.T       # E block, head j
                    bt[64 + 32 * j:64 + 32 * j + 32, :] = rot_o.T  # O block
                m[name] = np.ascontiguousarray(bt.astype(bf))
        in_maps.append(m)
    return in_maps, qk_bias


def kernel(x, mask, bias, rope_freqs, Wq, bq, Wk, bk, Wv, bv, Wo, bo, **extra):
    x = np.asarray(x); mask = np.asarray(mask); bias = np.asarray(bias)
    rope_freqs = np.asarray(rope_freqs)
    Wq = np.asarray(Wq); bq = np.asarray(bq); Wk = np.asarray(Wk); bk = np.asarray(bk)
    Wv = np.asarray(Wv); bv = np.asarray(bv); Wo = np.asarray(Wo); bo = np.asarray(bo)

    causal = bool(np.array_equal(mask != 0, np.tril(np.ones((S, S), dtype=bool))))
    if causal and _is_alibi(bias):
        in_maps, qk_bias = _host_prep_fast(x, rope_freqs, Wq, bq, Wk, bk, Wv, bv, Wo, bo)
        nc = _get_kernel_fast(qk_bias)
        res = run_bass_kernel_spmd(nc, in_maps, list(range(NCORES)))
        acc = np.zeros((R, D), dtype=np.float32)
        for c in range(NCORES):
            acc += res.results[c]["out"].astype(np.float32)
    else:
        in_maps, qk_bias = _host_prep(x, mask, bias, rope_freqs, Wq, bq, Wk, bk, Wv, bv,
                                      Wo, bo, causal)
        nc = _get_kernel(causal, qk_bias)
        res = run_bass_kernel_spmd(nc, in_maps, list(range(NCORES)))
        acc = np.zeros((R, D), dtype=np.float32)
        for c in range(NCORES):
            acc += res.results[c]["out"]
    acc += bo.astype(np.float32)[None, :]
    if np.any(bv):
        acc += (bv.astype(np.float32) @ Wo.T.astype(np.float32))[None, :]
    return acc.reshape(B, S, D).astype(np.float32)



# revision 32
# speedup vs baseline: 1.1680x; 1.1680x over previous
"""Trainium2 Bass kernel for nn_MultiHeadAttention (B=2,S=2048,D=1024,H=16, RoPE+ALiBi+causal).

Strategy: head-parallel sharding across 8 NeuronCores (2 heads/core, both batches).
Host folds mask+bias into a per-head multiplicative exp(bias) tensor (bf16),
pre-transposes x, and sums the per-core partial output projections.
Matmul datapath is bf16 (fp32 PSUM accumulation); output partials are fp32.
"""
import sys, os

for _p in ("/root/.axon_site/_ro/trn_rl_repo", "/opt/trn_rl_repo"):
    if os.path.isdir(_p) and _p not in sys.path:
        sys.path.insert(0, _p)

import numpy as np
import ml_dtypes
import concourse.bass as bass
import concourse.mybir as mybir
import concourse.tile as tile
from concourse import bacc
from concourse.bass_utils import run_bass_kernel_spmd

F32 = mybir.dt.float32
BF16 = mybir.dt.bfloat16

# Route Exp/Ln/Copy/Identity to the one table set containing them all, so the
# scalar engine never reloads activation tables mid-kernel (each reload ~1.3us).
_COMBINED_SET = "natural_log_exp_and_others"
_SHARED_FNS = {
    mybir.ActivationFunctionType.Exp, mybir.ActivationFunctionType.Ln,
    mybir.ActivationFunctionType.Copy, mybir.ActivationFunctionType.Identity,
}
_orig_get_act_tables = bacc.get_activation_tables


def _patched_get_act_tables(arch):
    t = _orig_get_act_tables(arch)
    if _COMBINED_SET in t and _SHARED_FNS <= t[_COMBINED_SET]:
        t = {name: (fns if name == _COMBINED_SET else fns - _SHARED_FNS)
             for name, fns in t.items()}
    return t


bacc.get_activation_tables = _patched_get_act_tables

B, S, D, H = 2, 2048, 1024, 16
DK = D // H            # 64
NCORES = 8
HPC = H // NCORES      # 2 heads per core
FD = HPC * DK          # 128 ctx features per core
R = B * S              # 4096 token rows
RT = R // 128          # 32 r-tiles
QB = 512               # q-block size
NQB = S // QB          # 4 q-blocks per batch


def _build(causal: bool, qk_bias: bool, stage: int = 2):
    """Build + compile the per-core SPMD kernel. Returns the compiled Bacc."""
    nc = bacc.Bacc()

    xT = nc.dram_tensor("xT", (D, R), BF16, kind="ExternalInput")
    wcat = nc.dram_tensor("wcat", (D, 3 * FD), BF16, kind="ExternalInput")
    woB = nc.dram_tensor("woB", (2 * DK, D), BF16, kind="ExternalInput")
    expb = nc.dram_tensor("expb", (HPC, S, S), BF16, kind="ExternalInput")
    cosp = nc.dram_tensor("cosp", (128, RT * (DK // 2)), BF16, kind="ExternalInput")
    sinp = nc.dram_tensor("sinp", (128, RT * (DK // 2)), BF16, kind="ExternalInput")
    ident = nc.dram_tensor("ident", (128, 128), BF16, kind="ExternalInput")
    onesd = nc.dram_tensor("onesd", (128, 64), F32, kind="ExternalInput")
    onesb = nc.dram_tensor("onesb", (128, 1), BF16, kind="ExternalInput")
    if qk_bias:
        bropeq = nc.dram_tensor("bropeq", (128, R), BF16, kind="ExternalInput")
        bropek = nc.dram_tensor("bropek", (128, R), BF16, kind="ExternalInput")
    out = nc.dram_tensor("out", (R, D), F32, kind="ExternalOutput")

    with tile.TileContext(nc) as tc:
        import contextlib
        ctx = contextlib.ExitStack()
        with ctx:
            consts = ctx.enter_context(tc.tile_pool(name="consts", bufs=1))
            persist = ctx.enter_context(tc.tile_pool(name="persist", bufs=1))

            # --- constants ---
            id_sb = consts.tile([128, 128], BF16)
            nc.sync.dma_start(out=id_sb, in_=ident[:, :])
            wc_sb = [consts.tile([128, 3 * FD], BF16, tag=f"wc{ct}", name=f"wc{ct}") for ct in range(8)]
            for ct in range(8):
                nc.sync.dma_start(out=wc_sb[ct], in_=wcat[ct * 128:(ct + 1) * 128, :])
            wo_sb = consts.tile([2 * DK, D], BF16, tag="wo", name="wo")
            nc.sync.dma_start(out=wo_sb, in_=woB[:, :])
            cos_sb = consts.tile([128, RT * 32], BF16)
            sin_sb = consts.tile([128, RT * 32], BF16)
            nc.sync.dma_start(out=cos_sb, in_=cosp[:, :])
            nc.sync.dma_start(out=sin_sb, in_=sinp[:, :])
            ones_sb = consts.tile([128, DK], F32)
            nc.sync.dma_start(out=ones_sb, in_=onesd[:, :])

            # --- persistent activation tensors ---
            QT = persist.tile([128, R], BF16, tag="QT")   # rows: h*64 + d, cols: b*2048+s
            KT = persist.tile([128, R], BF16, tag="KT")
            vaug = {}
            for rt in range(RT):
                for hh in range(HPC):
                    vaug[(rt, hh)] = persist.tile([128, DK + 1], BF16, tag=f"va{rt}_{hh}", name=f"va{rt}_{hh}")

            # =================== Phase 1: QKV projection + RoPE + transposes ===================
            with tc.tile_pool(name="p1x", bufs=3) as p1x, \
                 tc.tile_pool(name="p1n", bufs=1) as p1n, \
                 tc.tile_pool(name="p1s", bufs=4) as p1s, \
                 tc.tile_pool(name="p1ps", bufs=3, space="PSUM") as p1ps, \
                 tc.tile_pool(name="p1pt", bufs=3, space="PSUM") as p1pt:

                qknat = p1n.tile([128, RT * 256], BF16)  # col = rt*256 + {0:128 Q | 128:256 K}, d-major
                qkrot = p1n.tile([128, RT * 256], BF16)

                GRP = 8  # r-tiles per rope group
                for g in range(RT // GRP):
                    for rt in range(g * GRP, (g + 1) * GRP):
                        xts = [p1x.tile([128, 128], BF16, tag=f"x{ct}", name=f"xt{rt}_{ct}") for ct in range(8)]
                        for ct in range(8):
                            nc.sync.dma_start(
                                out=xts[ct],
                                in_=xT[ct * 128:(ct + 1) * 128, rt * 128:(rt + 1) * 128])
                        pp = p1ps.tile([128, 3 * FD], F32, tag="prj")
                        for ct in range(8):
                            nc.tensor.matmul(pp, xts[ct], wc_sb[ct],
                                             start=(ct == 0), stop=(ct == 7))
                        # drain Q,K -> qknat (bf16); V -> vaug tiles (bf16)
                        nc.vector.tensor_copy(qknat[:, rt * 256: rt * 256 + 256], pp[:, 0:256])
                        for hh in range(HPC):
                            va = vaug[(rt, hh)]
                            nc.scalar.copy(va[:, 0:DK], pp[:, 2 * FD + hh * DK: 2 * FD + (hh + 1) * DK])
                            nc.sync.dma_start(out=va[:, DK:DK + 1], in_=onesb[:, 0:1])

                    # rope on group g: Q and K separately, 6 ops each
                    # AP dims over qknat: [part][rt(8) step 256][head(2) step 64][pair(32) step 2]
                    def sl(t, qk, eo, g=g):
                        a = t[:, :]
                        return bass.AP(
                            tensor=a.tensor,
                            offset=a.offset + (g * GRP * 256 + qk * 128 + eo),
                            ap=[a.ap[0], [256, GRP], [64, 2], [2, 32]],
                        )
                    def slc(t, g=g):  # cos/sin: [part][rt(8) step 32][head 0x2][pair step 1 x32]
                        a = t[:, :]
                        return bass.AP(
                            tensor=a.tensor,
                            offset=a.offset + g * GRP * 32,
                            ap=[a.ap[0], [32, GRP], [0, 2], [1, 32]],
                        )
                    for qk in range(2):
                        s1 = p1s.tile([128, GRP * 64], BF16, tag="s1")
                        s2 = p1s.tile([128, GRP * 64], BF16, tag="s2")
                        s3 = p1s.tile([128, GRP * 64], BF16, tag="s3")
                        s4 = p1s.tile([128, GRP * 64], BF16, tag="s4")
                        nc.vector.tensor_mul(s1, sl(qknat, qk, 0), slc(cos_sb))
                        nc.vector.tensor_mul(s2, sl(qknat, qk, 1), slc(sin_sb))
                        nc.vector.tensor_sub(sl(qkrot, qk, 0), s1, s2)
                        nc.vector.tensor_mul(s3, sl(qknat, qk, 0), slc(sin_sb))
                        nc.vector.tensor_mul(s4, sl(qknat, qk, 1), slc(cos_sb))
                        nc.vector.tensor_add(sl(qkrot, qk, 1), s3, s4)

                    # transposes for the PREVIOUS group (skewed so PE never stalls on rope)
                    if g > 0:
                        for rt in range((g - 1) * GRP, g * GRP):
                            for qk, dst in ((0, QT), (1, KT)):
                                pt = p1pt.tile([128, 128], BF16, tag="pt")
                                nc.tensor.transpose(pt, qkrot[:, rt * 256 + qk * 128: rt * 256 + qk * 128 + 128], id_sb)
                                nc.scalar.copy(dst[:, rt * 128:(rt + 1) * 128], pt)

                for rt in range(RT - GRP, RT):
                    for qk, dst in ((0, QT), (1, KT)):
                        pt = p1pt.tile([128, 128], BF16, tag="pt")
                        nc.tensor.transpose(pt, qkrot[:, rt * 256 + qk * 128: rt * 256 + qk * 128 + 128], id_sb)
                        nc.scalar.copy(dst[:, rt * 128:(rt + 1) * 128], pt)

                if qk_bias:
                    brq = p1n.tile([128, R], BF16, tag="brq")
                    brk = p1n.tile([128, R], BF16, tag="brk")
                    nc.sync.dma_start(out=brq, in_=bropeq[:, :])
                    nc.sync.dma_start(out=brk, in_=bropek[:, :])
                    nc.vector.tensor_add(QT, QT, brq)
                    nc.vector.tensor_add(KT, KT, brk)

            if stage == 1:
                for i in range(4):
                    nc.gpsimd.dma_start(out=out[i * 128:(i + 1) * 128, :],
                                        in_=QT[:, i * 1024:(i + 1) * 1024])
                    nc.gpsimd.dma_start(out=out[512 + i * 128: 512 + (i + 1) * 128, :],
                                        in_=KT[:, i * 1024:(i + 1) * 1024])
                nc.gpsimd.dma_start(out=out[1024:1152, 0:65], in_=vaug[(0, 0)])
                nc.gpsimd.dma_start(out=out[1152:1280, 0:65], in_=vaug[(31, 1)])
            # =================== Phase 2: attention + output projection ===================
            if stage >= 2:
              with tc.tile_pool(name="p2eb", bufs=4) as p2eb, \
                 tc.tile_pool(name="p2e", bufs=3) as p2e, \
                 tc.tile_pool(name="p2a", bufs=6) as p2a, \
                 tc.tile_pool(name="p2c", bufs=2) as p2c, \
                 tc.tile_pool(name="p2o", bufs=3) as p2o, \
                 tc.tile_pool(name="psc", bufs=2, space="PSUM") as psc, \
                 tc.tile_pool(name="psx", bufs=1, space="PSUM") as psx, \
                 tc.tile_pool(name="pm", bufs=1, space="PSUM") as pm:

                for qb in range(NQB):
                    nkt = (qb + 1) * (QB // 128) if causal else S // 128
                    csc = {}
                    for b in range(B):
                        csc[b] = p2c.tile([2 * DK, QB], BF16, tag=f"cb{b}", name=f"cb{qb}{b}")
                    for hh in range(HPC):
                        ctx_ps = {}
                        for b in range(B):
                            ctx_ps[b] = psx.tile([DK + 1, QB], F32, tag=f"ctx{b}", name=f"ctx{qb}_{hh}{b}")
                        pend = None  # (kt, q_off, q_len, at_tiles) awaiting ctx issue
                        def issue_ctx(pkt, p_off, p_len, p_at):
                            for b in range(B):
                                nc.tensor.matmul(
                                    ctx_ps[b][:, p_off:QB],
                                    vaug[(b * (S // 128) + pkt, hh)],
                                    p_at[b][:, 0:p_len],
                                    start=(pkt == 0), stop=(pkt == nkt - 1))
                        for kt in range(nkt):
                            q_off = max(0, kt * 128 - qb * QB) if causal else 0
                            q_len = QB - q_off
                            ebt = p2eb.tile([128, QB], BF16, tag="eb", name=f"eb{qb}_{hh}_{kt}")
                            nc.sync.dma_start(
                                out=ebt[:, 0:q_len],
                                in_=expb[hh, kt * 128:(kt + 1) * 128,
                                         qb * QB + q_off: (qb + 1) * QB])
                            scp = psc.tile([128, 2 * QB], F32, tag="sc", name=f"sc{qb}_{hh}_{kt}")
                            for b in range(B):
                                nc.tensor.matmul(
                                    scp[:, b * QB: b * QB + q_len],
                                    KT[hh * DK:(hh + 1) * DK, b * S + kt * 128: b * S + (kt + 1) * 128],
                                    QT[hh * DK:(hh + 1) * DK, b * S + qb * QB + q_off: b * S + (qb + 1) * QB],
                                    start=True, stop=True)
                            if pend is not None:
                                issue_ctx(*pend)
                            ex = p2e.tile([128, 2 * QB], BF16, tag="ex", name=f"ex{qb}_{hh}_{kt}")
                            nc.scalar.activation(ex[:, 0:QB + q_len], scp[:, 0:QB + q_len],
                                                 mybir.ActivationFunctionType.Exp)
                            at_tiles = []
                            for b in range(B):
                                at = p2a.tile([128, QB], BF16, tag=f"at{b}", name=f"at{qb}_{hh}_{kt}_{b}")
                                nc.vector.tensor_mul(at[:, 0:q_len], ex[:, b * QB: b * QB + q_len],
                                                     ebt[:, 0:q_len])
                                at_tiles.append(at)
                            pend = (kt, q_off, q_len, at_tiles)
                        issue_ctx(*pend)
                        # normalize this head into csc halves (frees ctx banks for next head)
                        for b in range(B):
                            cp = ctx_ps[b]
                            dsb = p2c.tile([DK + 1, QB], F32, tag=f"ds{b}", name=f"ds{qb}{hh}{b}")
                            nc.vector.tensor_copy(dsb[DK:DK + 1, :], cp[DK:DK + 1, :])
                            # transpose den row across partitions: dT[p, j] = den[p + 128 j]
                            dT = p2c.tile([128, 4], F32, tag=f"dT{b}", name=f"dT{qb}{hh}{b}")
                            a = dsb[DK:DK + 1, :]
                            for j in range(4):
                                nc.sync.dma_start(
                                    out=dT[:, j:j + 1],
                                    in_=bass.AP(tensor=a.tensor, offset=a.offset + 128 * j,
                                                ap=[a.ap[0], [1, 128]]))
                            rT = p2c.tile([128, 4], F32, tag=f"rT{b}", name=f"rT{qb}{hh}{b}")
                            nc.vector.reciprocal(rT, dT)
                            rc = p2c.tile([DK + 1, QB], F32, tag=f"rc{b}", name=f"rc{qb}{hh}{b}")
                            rca = rc[DK:DK + 1, :]
                            for j in range(4):
                                nc.sync.dma_start(
                                    out=bass.AP(tensor=rca.tensor, offset=rca.offset + 128 * j,
                                                ap=[rca.ap[0], [1, 128]]),
                                    in_=rT[:, j:j + 1])
                            rbp = pm.tile([DK, QB], F32, tag="rb")
                            nc.tensor.matmul(rbp, ones_sb[DK:DK + 1, :], rc[DK:DK + 1, :],
                                             start=True, stop=True)
                            rb = p2c.tile([DK, QB], F32, tag=f"rb{b}", name=f"rb{qb}{hh}{b}")
                            nc.vector.tensor_copy(rb, rbp)
                            if hh == 0:
                                nc.vector.tensor_mul(csc[b][0:DK, :], cp[0:DK, :], rb)
                            else:
                                cs1 = p2c.tile([DK, QB], BF16, tag=f"cs{b}", name=f"cs{qb}{hh}{b}")
                                nc.vector.tensor_mul(cs1, cp[0:DK, :], rb)
                                nc.sync.dma_start(out=csc[b][DK:2 * DK, :], in_=cs1)
                    for b in range(B):
                        for rs in range(QB // 128):
                            ot = p2o.tile([128, D], F32, tag="ot")
                            for eh in range(2):
                                op = pm.tile([128, 512], F32, tag="op", name=f"op{qb}{b}{rs}{eh}")
                                nc.tensor.matmul(op, csc[b][:, rs * 128:(rs + 1) * 128],
                                                 wo_sb[:, eh * 512:(eh + 1) * 512],
                                                 start=True, stop=True)
                                if eh == 0:
                                    nc.vector.tensor_copy(ot[:, 0:512], op)
                                else:
                                    nc.scalar.copy(ot[:, 512:1024], op)
                            nc.sync.dma_start(
                                out=out[b * S + qb * QB + rs * 128: b * S + qb * QB + (rs + 1) * 128, :],
                                in_=ot)
    nc.compile()
    return nc


def _build_fast(qk_bias: bool):
    """Causal+ALiBi specialized kernel.

    Head slots per core: hh=0 steep head (slope>=2^-4), banded to the last 8
    k-tiles; hh=1 shallow head (slope<=2^-4.5), ALiBi applied via a
    per-partition linear bias folded into the exp (the per-query factor
    cancels in softmax normalization), so its off-diagonal tiles need no
    elementwise multiply at all.
    """
    nc = bacc.Bacc()

    xp = nc.dram_tensor("xp", (128, RT * 1024), BF16, kind="ExternalInput")
    wc2 = nc.dram_tensor("wc2", (128, 8 * 384), BF16, kind="ExternalInput")
    woB = nc.dram_tensor("woB", (128, D), BF16, kind="ExternalInput")
    cosp = nc.dram_tensor("cosp", (128, RT * 64), BF16, kind="ExternalInput")
    sinp = nc.dram_tensor("sinp", (128, RT * 64), BF16, kind="ExternalInput")
    ident = nc.dram_tensor("ident", (128, 128), BF16, kind="ExternalInput")
    onesd = nc.dram_tensor("onesd", (128, 64), F32, kind="ExternalInput")
    eual = nc.dram_tensor("eual", (128, 3 * 512), BF16, kind="ExternalInput")
    biasv = nc.dram_tensor("biasv", (128, 40), F32, kind="ExternalInput")
    if qk_bias:
        bropeq = nc.dram_tensor("bropeq", (128, R), BF16, kind="ExternalInput")
        bropek = nc.dram_tensor("bropek", (128, R), BF16, kind="ExternalInput")
    out = nc.dram_tensor("out", (R, D), BF16, kind="ExternalOutput")

    with tile.TileContext(nc) as tc:
        import contextlib
        ctx = contextlib.ExitStack()
        with ctx:
            consts = ctx.enter_context(tc.tile_pool(name="consts", bufs=1))
            persist = ctx.enter_context(tc.tile_pool(name="persist", bufs=1))

            # --- constants ---
            id_sb = consts.tile([128, 128], BF16)
            nc.scalar.dma_start(out=id_sb, in_=ident[:, :])
            wc_sb = consts.tile([128, 8 * 384], BF16, tag="wc", name="wc")
            nc.scalar.dma_start(out=wc_sb, in_=wc2[:, :])
            cos_sb = consts.tile([128, RT * 64], BF16)
            sin_sb = consts.tile([128, RT * 64], BF16)
            nc.scalar.dma_start(out=cos_sb, in_=cosp[:, :])
            nc.scalar.dma_start(out=sin_sb, in_=sinp[:, :])
            ones_sb = consts.tile([128, DK], F32)
            nc.scalar.dma_start(out=ones_sb, in_=onesd[:, :])
            bv_sb = consts.tile([128, 40], F32, tag="bv", name="bv")
            nc.scalar.dma_start(out=bv_sb, in_=biasv[:, :])
            # needed only in phase 2 — loaded late so x tiles go first
            wo_sb = consts.tile([128, D], BF16, tag="wo", name="wo")
            eu_sb = consts.tile([128, 3 * 512], BF16, tag="eu", name="eu")

            # --- persistent activation tensors ---
            # QT/KT rows (after E/O-deinterleaved projection):
            #   [Qe_h0 0:32 | Qe_h1 32:64 | Qo_h0 64:96 | Qo_h1 96:128]
            QT = persist.tile([128, R], BF16, tag="QT")
            KT = persist.tile([128, R], BF16, tag="KT")
            # per-head dup tiles: rows 0:64 = [e|o] of head hh (for batch 0 / T0),
            # rows 64:128 = same (for batch 1 / T8)
            QTd = [persist.tile([128, R], BF16, tag=f"QTd{hh}", name=f"QTd{hh}") for hh in range(2)]
            KTd = [persist.tile([128, R], BF16, tag=f"KTd{hh}", name=f"KTd{hh}") for hh in range(2)]

            def dup_group(g, GRP=8):
                c0, c1 = g * GRP * 128, (g + 1) * GRP * 128
                for src, dsts in ((QT, QTd), (KT, KTd)):
                    for hh in range(2):
                        for half in range(2):
                            for eo in range(2):
                                nc.gpsimd.dma_start(
                                    out=dsts[hh][half * 64 + eo * 32: half * 64 + eo * 32 + 32, c0:c1],
                                    in_=src[eo * 64 + hh * 32: eo * 64 + hh * 32 + 32, c0:c1])
            # vaug: one tile; slice (rt, hh) at col (rt*2+hh)*65, 65 wide
            vaug = persist.tile([128, RT * 2 * 65], BF16, tag="va", name="va")
            va_ones = bass.AP(tensor=vaug[:, :].tensor,
                              offset=vaug[:, :].offset + 64,
                              ap=[vaug[:, :].ap[0], [65, RT * 2]])
            nc.vector.memset(va_ones, 1.0)

            def va_sl(rt, hh):
                return vaug[:, (rt * 2 + hh) * 65:(rt * 2 + hh) * 65 + 65]

            # ============ Phase 1: QKV projection + RoPE + transposes ============
            with tc.tile_pool(name="p1x", bufs=3) as p1x, \
                 tc.tile_pool(name="p1n", bufs=1) as p1n, \
                 tc.tile_pool(name="p1s", bufs=4) as p1s, \
                 tc.tile_pool(name="p1ps", bufs=3, space="PSUM") as p1ps, \
                 tc.tile_pool(name="p1w", bufs=1, space="PSUM") as p1w, \
                 tc.tile_pool(name="p1pt", bufs=3, space="PSUM") as p1pt:

                qknat = p1n.tile([128, RT * 256], BF16)
                qkrot = p1n.tile([128, RT * 256], BF16)

                # HAM warm-up: ~4us of dummy matmuls on the identity tile while
                # the first x tiles stream in, so projection starts at full clock
                warm_ps = p1w.tile([128, 128], F32, tag="warm", name="warm")
                for _w in range(36):
                    nc.tensor.matmul(warm_ps, id_sb, id_sb, start=(_w == 0), stop=(_w == 35))

                GRP = 8

                def do_transpose(rt):
                    for qk, dst in ((0, QT), (1, KT)):
                        pt = p1pt.tile([128, 128], BF16, tag="pt")
                        nc.tensor.transpose(pt, qkrot[:, rt * 256 + qk * 128: rt * 256 + qk * 128 + 128], id_sb)
                        if qk == 0:
                            nc.scalar.copy(dst[:, rt * 128:(rt + 1) * 128], pt)
                        else:
                            nc.vector.tensor_copy(dst[:, rt * 128:(rt + 1) * 128], pt)

                for g in range(RT // GRP):
                    for rt in range(g * GRP, (g + 1) * GRP):
                        xts = p1x.tile([128, 1024], BF16, tag="x", name=f"xt{rt}")
                        nc.sync.dma_start(out=xts, in_=xp[:, rt * 1024:(rt + 1) * 1024])
                        pp = p1ps.tile([128, 3 * FD], F32, tag="prj")
                        for ct in range(8):
                            nc.tensor.matmul(pp, xts[:, ct * 128:(ct + 1) * 128],
                                             wc_sb[:, ct * 384:(ct + 1) * 384],
                                             start=(ct == 0), stop=(ct == 7))
                        if g > 0:
                            do_transpose(rt - GRP)
                        nc.vector.tensor_copy(qknat[:, rt * 256: rt * 256 + 256], pp[:, 0:256])
                        # V drain: one op covers both heads (65-strided dest)
                        vd = va_sl(rt, 0)[:, 0:DK]
                        vdst = bass.AP(tensor=vd.tensor, offset=vd.offset,
                                       ap=[vd.ap[0], [65, 2], [1, DK]])
                        vsrc_a = pp[:, 2 * FD: 2 * FD + 128]
                        vsrc = bass.AP(tensor=vsrc_a.tensor, offset=vsrc_a.offset,
                                       ap=[vsrc_a.ap[0], [64, 2], [1, DK]])
                        nc.scalar.copy(vdst, vsrc)

                    def sl(t, qk, eo, g=g):
                        a = t[:, :]
                        return bass.AP(
                            tensor=a.tensor,
                            offset=a.offset + (g * GRP * 256 + qk * 128 + eo * 64),
                            ap=[a.ap[0], [256, GRP], [1, 64]],
                        )
                    def slc(t, g=g):
                        a = t[:, :]
                        return bass.AP(
                            tensor=a.tensor,
                            offset=a.offset + g * GRP * 64,
                            ap=[a.ap[0], [64, GRP], [1, 64]],
                        )
                    for qk in range(2):
                        s1 = p1s.tile([128, GRP * 64], BF16, tag="s1")
                        s2 = p1s.tile([128, GRP * 64], BF16, tag="s2")
                        s3 = p1s.tile([128, GRP * 64], BF16, tag="s3")
                        s4 = p1s.tile([128, GRP * 64], BF16, tag="s4")
                        nc.vector.tensor_mul(s1, sl(qknat, qk, 0), slc(cos_sb))
                        nc.vector.tensor_mul(s2, sl(qknat, qk, 1), slc(sin_sb))
                        nc.vector.tensor_sub(sl(qkrot, qk, 0), s1, s2)
                        nc.vector.tensor_mul(s3, sl(qknat, qk, 0), slc(sin_sb))
                        nc.vector.tensor_mul(s4, sl(qknat, qk, 1), slc(cos_sb))
                        nc.vector.tensor_add(sl(qkrot, qk, 1), s3, s4)

                    if g == 0:
                        nc.scalar.dma_start(out=wo_sb, in_=woB[:, :])
                        nc.scalar.dma_start(out=eu_sb, in_=eual[:, :])
                    if g > 0 and not qk_bias:
                        dup_group(g - 1)

                for rt in range(RT - GRP, RT):
                    do_transpose(rt)
                if not qk_bias:
                    dup_group(RT // GRP - 1)

                if qk_bias:
                    brq = p1n.tile([128, R], BF16, tag="brq")
                    brk = p1n.tile([128, R], BF16, tag="brk")
                    nc.sync.dma_start(out=brq, in_=bropeq[:, :])
                    nc.sync.dma_start(out=brk, in_=bropek[:, :])
                    nc.vector.tensor_add(QT, QT, brq)
                    nc.vector.tensor_add(KT, KT, brk)
                    for g in range(RT // GRP):
                        dup_group(g)

            # ============ Phase 2: attention + output projection ============
            with tc.tile_pool(name="p2e", bufs=3) as p2e, \
                 tc.tile_pool(name="p2a", bufs=3) as p2a, \
                 tc.tile_pool(name="p2c", bufs=2) as p2c, \
                 tc.tile_pool(name="p2o", bufs=3) as p2o, \
                 tc.tile_pool(name="psc", bufs=2, space="PSUM") as psc, \
                 tc.tile_pool(name="psx", bufs=1, space="PSUM") as psx, \
                 tc.tile_pool(name="pm", bufs=1, space="PSUM") as pm:

                def seg_ap(t, q_len, off=0):
                    a = t[:, :]
                    return bass.AP(tensor=a.tensor, offset=a.offset + off,
                                   ap=[a.ap[0], [QB, 2], [1, q_len]])

                # deferred norm/outproj steps, interleaved into later tile loops so
                # the PE stream never stalls on a normalization dependency chain
                pending = []

                def drain_steps(k):
                    for _ in range(min(k, len(pending))):
                        pending.pop(0)()

                def norm_steps(qb, hh, cp, csc_b, b):
                    st = {}
                    def s1():
                        dsb = p2c.tile([DK + 1, QB], F32, tag=f"ds{b}", name=f"ds{qb}{hh}{b}")
                        nc.vector.tensor_copy(dsb[DK:DK + 1, :], cp[DK:DK + 1, :])
                        lnr = p2c.tile([DK + 1, QB], F32, tag=f"ln{b}", name=f"ln{qb}{hh}{b}")
                        nc.scalar.activation(lnr[DK:DK + 1, :], dsb[DK:DK + 1, :],
                                             mybir.ActivationFunctionType.Ln)
                        st["ln"] = lnr
                    def s2():
                        lnr = st["ln"]
                        rbp = pm.tile([DK, QB], F32, tag="rb")
                        nc.tensor.matmul(rbp, ones_sb[DK:DK + 1, :], lnr[DK:DK + 1, :],
                                         start=True, stop=True)
                        rb = p2c.tile([DK, QB], F32, tag=f"rb{b}", name=f"rb{qb}{hh}{b}")
                        nc.scalar.activation(rb, rbp, mybir.ActivationFunctionType.Exp,
                                             scale=-1.0)
                        st["rb"] = rb
                    def s3():
                        rb = st["rb"]
                        if hh == 0:
                            nc.vector.tensor_mul(csc_b[0:DK, :], cp[0:DK, :], rb)
                        else:
                            cs1 = p2c.tile([DK, QB], BF16, tag=f"cs{b}", name=f"cs{qb}{hh}{b}")
                            nc.vector.tensor_mul(cs1, cp[0:DK, :], rb)
                            nc.sync.dma_start(out=csc_b[DK:2 * DK, :], in_=cs1)
                    return [s1, s2, s3]

                def outproj_steps(qb, csc):
                    steps = []
                    for b in range(B):
                        for rs in range(QB // 128):
                            def st(b=b, rs=rs, qb=qb, csc=csc):
                                ot = p2o.tile([128, D], BF16, tag="ot")
                                for eh in range(2):
                                    op = pm.tile([128, 512], F32, tag="op", name=f"op{qb}{b}{rs}{eh}")
                                    nc.tensor.matmul(op, csc[b][:, rs * 128:(rs + 1) * 128],
                                                     wo_sb[:, eh * 512:(eh + 1) * 512],
                                                     start=True, stop=True)
                                    if eh == 0:
                                        nc.vector.tensor_copy(ot[:, 0:512], op)
                                    else:
                                        nc.vector.tensor_copy(ot[:, 512:1024], op)
                                nc.gpsimd.dma_start(
                                    out=out[b * S + qb * QB + rs * 128: b * S + qb * QB + (rs + 1) * 128, :],
                                    in_=ot)
                            steps.append(st)
                    return steps

                csc_all = {}
                for qb in range(NQB):
                    nkt = (qb + 1) * 4
                    csc = {}
                    for b in range(B):
                        csc[b] = p2c.tile([2 * DK, QB], BF16, tag=f"cb{b}", name=f"cb{qb}{b}")
                    csc_all[qb] = csc
                    for hh in ((1, 0) if qb == NQB - 1 else (0, 1)):
                        kt_lo = max(0, nkt - 6) if hh == 0 else 0
                        ctx_ps = {}
                        for b in range(B):
                            ctx_ps[b] = psx.tile([DK + 1, QB], F32, tag=f"ctx{b}", name=f"ctx{qb}_{hh}{b}")
                        pend = None
                        def issue_ctx(pkt, p_off, p_len, p_rhs, kt_lo=kt_lo, nkt=nkt, hh=hh, ctx_ps=ctx_ps):
                            for b in range(B):
                                nc.tensor.matmul(
                                    ctx_ps[b][:, p_off:QB],
                                    va_sl(b * (S // 128) + pkt, hh),
                                    p_rhs[:, b * QB: b * QB + p_len],
                                    start=(pkt == kt_lo), stop=(pkt == nkt - 1))
                        for kt in range(kt_lo, nkt):
                            diag = kt >= 4 * qb
                            q_off = max(0, kt * 128 - qb * QB)
                            q_len = QB - q_off
                            m = 0 if diag else 4 * qb - kt
                            k_idx = m if hh == 0 else m - (q_off // 128)
                            col = hh * 20 + k_idx + 3
                            scp = psc.tile([128, 2 * QB], F32, tag="sc", name=f"sc{qb}_{hh}_{kt}")
                            for b in range(B):
                                # b=0 on PE row-group 0, b=1 on row-group 64: concurrent
                                nc.tensor.matmul(
                                    scp[:, b * QB: b * QB + q_len],
                                    KTd[hh][64 * b:64 * b + 64, b * S + kt * 128: b * S + (kt + 1) * 128],
                                    QTd[hh][64 * b:64 * b + 64, b * S + qb * QB + q_off: b * S + (qb + 1) * QB],
                                    start=True, stop=True, tile_position=(64 * b, 0))
                            if pend is not None:
                                issue_ctx(*pend)
                            drain_steps(2)
                            ex = p2e.tile([128, 2 * QB], BF16, tag="ex", name=f"ex{qb}_{hh}_{kt}")
                            nc.scalar.activation(seg_ap(ex, q_len), seg_ap(scp, q_len),
                                                 mybir.ActivationFunctionType.Exp,
                                                 bias=bv_sb[:, col:col + 1])
                            if hh == 0 or diag:
                                # eu blocks: [EU0 | EM0 | Mbin]
                                blk = (1 if diag else 0) if hh == 0 else 2
                                ea = eu_sb[:, blk * QB: blk * QB + QB]
                                eap = bass.AP(tensor=ea.tensor, offset=ea.offset,
                                              ap=[ea.ap[0], [0, 2], [1, q_len]])
                                at = p2a.tile([128, 2 * QB], BF16, tag="at", name=f"at{qb}_{hh}_{kt}")
                                nc.vector.tensor_mul(seg_ap(at, q_len), seg_ap(ex, q_len), eap)
                                rhs_t = at
                            else:
                                rhs_t = ex
                            pend = (kt, q_off, q_len, rhs_t)
                        issue_ctx(*pend)
                        for b in range(B):
                            pending.extend(norm_steps(qb, hh, ctx_ps[b], csc[b], b))
                    pending.extend(outproj_steps(qb, csc))
                while pending:
                    pending.pop(0)()
    nc.compile()
    return nc


_CACHE = {}


def _get_kernel(causal: bool, qk_bias: bool):
    key = (causal, qk_bias)
    if key not in _CACHE:
        _CACHE[key] = _build(causal, qk_bias)
    return _CACHE[key]


def _get_kernel_fast(qk_bias: bool):
    key = ("fast", qk_bias)
    if key not in _CACHE:
        _CACHE[key] = _build_fast(qk_bias)
    return _CACHE[key]


def _host_prep(x, mask, bias, rope_freqs, Wq, bq, Wk, bk, Wv, bv, Wo, bo, causal):
    """Build the 8 per-core input maps."""
    bf = ml_dtypes.bfloat16
    xf = np.ascontiguousarray(x.reshape(R, D).T.astype(bf))  # (D, R)
    cosf = np.cos(rope_freqs.astype(np.float32))  # (S, 32)
    sinf = np.sin(rope_freqs.astype(np.float32))
    rr = np.arange(R)
    cs_full = cosf[rr % S]  # (R, 32)
    sn_full = sinf[rr % S]
    cosp = np.ascontiguousarray(
        cs_full.reshape(RT, 128, 32).transpose(1, 0, 2).reshape(128, RT * 32).astype(bf))
    sinp = np.ascontiguousarray(
        sn_full.reshape(RT, 128, 32).transpose(1, 0, 2).reshape(128, RT * 32).astype(bf))
    identm = np.eye(128, dtype=np.float32).astype(bf)

    qk_bias = bool(np.any(bq) or np.any(bk))
    maskT = (mask != 0).T  # (k, q)

    in_maps = []
    for c in range(NCORES):
        h0 = c * HPC
        fsl = slice(c * FD, (c + 1) * FD)
        wq = Wq[fsl, :].astype(np.float32) / np.sqrt(np.float32(DK))
        wk = Wk[fsl, :].astype(np.float32)
        wv = Wv[fsl, :].astype(np.float32)
        wcat = np.ascontiguousarray(np.concatenate([wq, wk, wv], axis=0).T.astype(bf))  # (D, 384)
        wob = np.ascontiguousarray(Wo[:, c * FD: (c + 1) * FD].T.astype(bf))  # (128, D)
        eb = np.empty((HPC, S, S), dtype=bf)
        for j in range(HPC):
            bT = bias[h0 + j].T.astype(np.float32)  # (k, q)
            eb[j] = np.where(maskT, np.exp(np.minimum(bT, np.float32(80.0))),
                             np.float32(0)).astype(bf)
        m = {
            "xT": xf, "wcat": wcat, "woB": wob,
            "expb": eb, "cosp": cosp, "sinp": sinp, "ident": identm,
            "onesd": np.ones((128, 64), dtype=np.float32),
            "onesb": np.ones((128, 1), dtype=bf),
        }
        if qk_bias:
            for name, bvec in (("bropeq", bq / np.sqrt(np.float32(DK))), ("bropek", bk)):
                bt = np.empty((128, R), dtype=np.float32)
                bb = bvec[fsl].astype(np.float32).reshape(HPC, DK // 2, 2)
                for j in range(HPC):
                    be = bb[j, :, 0][None, :]  # (1, 32)
                    bo_ = bb[j, :, 1][None, :]
                    rot_e = be * cs_full - bo_ * sn_full   # (R, 32)
                    rot_o = be * sn_full + bo_ * cs_full
                    blk = np.empty((R, DK), dtype=np.float32)
                    blk[:, 0::2] = rot_e
                    blk[:, 1::2] = rot_o
                    bt[j * DK:(j + 1) * DK, :] = blk.T
                m[name] = np.ascontiguousarray(bt.astype(bf))
        in_maps.append(m)
    return in_maps, qk_bias


_SLOPES = 2.0 ** (-8.0 * (np.arange(1, H + 1) / H))  # float64, matches reference


def _is_alibi(bias):
    pos = np.arange(S)
    rel = (pos[None, :] - pos[:, None]).astype(np.float64)
    for h in range(H):
        if not np.allclose(bias[h], (_SLOPES[h] * rel).astype(np.float32),
                           rtol=1e-5, atol=1e-6):
            return False
    return True


def _host_prep_fast(x, rope_freqs, Wq, bq, Wk, bk, Wv, bv, Wo, bo):
    bf = ml_dtypes.bfloat16
    xp = np.ascontiguousarray(
        x.reshape(RT, 128, 8, 128).transpose(3, 0, 2, 1).reshape(128, RT * 1024)
        .astype(bf))
    cosf = np.cos(rope_freqs.astype(np.float32))
    sinf = np.sin(rope_freqs.astype(np.float32))
    rr = np.arange(R)
    cs_full = cosf[rr % S]
    sn_full = sinf[rr % S]
    # cos/sin duplicated per head: col rt*64 + j holds freq j%32 at token rt*128+p
    cs2 = np.concatenate([cs_full, cs_full], axis=1)  # (R, 64)
    sn2 = np.concatenate([sn_full, sn_full], axis=1)
    cosp = np.ascontiguousarray(
        cs2.reshape(RT, 128, 64).transpose(1, 0, 2).reshape(128, RT * 64).astype(bf))
    sinp = np.ascontiguousarray(
        sn2.reshape(RT, 128, 64).transpose(1, 0, 2).reshape(128, RT * 64).astype(bf))
    identm = np.eye(128, dtype=np.float32).astype(bf)
    qk_bias = bool(np.any(bq) or np.any(bk))

    pcol = np.arange(128, dtype=np.float64)[:, None]
    fcol = np.arange(QB, dtype=np.float64)[None, :]

    in_maps = []
    for c in range(NCORES):
        h0, h1 = c, 8 + c
        rows = np.r_[64 * h0:64 * h0 + 64, 64 * h1:64 * h1 + 64]
        # E/O-deinterleaved rows for Q,K: [h0-even, h1-even, h0-odd, h1-odd]
        ev, od = np.arange(0, 64, 2), np.arange(1, 64, 2)
        rows_eo = np.r_[64 * h0 + ev, 64 * h1 + ev, 64 * h0 + od, 64 * h1 + od]
        s0, s1 = _SLOPES[h0], _SLOPES[h1]
        wq = Wq[rows_eo, :].astype(np.float32) / np.sqrt(np.float32(DK))
        wk = Wk[rows_eo, :].astype(np.float32)
        wv = Wv[rows, :].astype(np.float32)
        wcat = np.concatenate([wq, wk, wv], axis=0).T  # (1024, 384)
        wc2 = np.ascontiguousarray(
            wcat.reshape(8, 128, 384).transpose(1, 0, 2).reshape(128, 8 * 384).astype(bf))
        wob = np.ascontiguousarray(Wo[:, rows].T.astype(bf))  # (128, 1024)
        eu0 = np.exp(s0 * (pcol - fcol - 64.0))
        em0 = np.where(pcol <= fcol, eu0, 0.0)
        mbin = (pcol <= fcol).astype(np.float64)
        eual = np.ascontiguousarray(
            np.concatenate([eu0, em0, mbin], axis=1).astype(np.float32).astype(bf))
        bv40 = np.zeros((128, 40), dtype=np.float32)
        for k in range(-3, 17):
            bv40[:, k + 3] = np.float32(s0 * (64.0 - 128.0 * k))
            bv40[:, 20 + k + 3] = (s1 * (pcol[:, 0] - 63.0 - 128.0 * k)).astype(np.float32)
        m = {
            "xp": xp, "wc2": wc2, "woB": wob,
            "cosp": cosp, "sinp": sinp, "ident": identm,
            "onesd": np.ones((128, 64), dtype=np.float32),
            "eual": eual, "biasv": bv40,
        }
        if qk_bias:
            for name, bvec in (("bropeq", bq / np.sqrt(np.float32(DK))), ("bropek", bk)):
                bt = np.empty((128, R), dtype=np.float32)
                bb = bvec[rows].astype(np.float32).reshape(HPC, DK // 2, 2)
                for j in range(HPC):
                    be = bb[j, :, 0][None, :]
                    bo_ = bb[j, :, 1][None, :]
                    rot_e = be * cs_full - bo_ * sn_full  # (R, 32)
                    rot_o = be * sn_full + bo_ * cs_full
                    bt[32 * j:32 * j + 32, :] = rot_e.T       # E block, head j
                    bt[64 + 32 * j:64 + 32 * j + 32, :] = rot_o.T  # O block
                m[name] = np.ascontiguousarray(bt.astype(bf))
        in_maps.append(m)
    return in_maps, qk_bias


def kernel(x, mask, bias, rope_freqs, Wq, bq, Wk, bk, Wv, bv, Wo, bo, **extra):
    x = np.asarray(x); mask = np.asarray(mask); bias = np.asarray(bias)
    rope_freqs = np.asarray(rope_freqs)
    Wq = np.asarray(Wq); bq = np.asarray(bq); Wk = np.asarray(Wk); bk = np.asarray(bk)
    Wv = np.asarray(Wv); bv = np.asarray(bv); Wo = np.asarray(Wo); bo = np.asarray(bo)

    causal = bool(np.array_equal(mask != 0, np.tril(np.ones((S, S), dtype=bool))))
    if causal and _is_alibi(bias):
        in_maps, qk_bias = _host_prep_fast(x, rope_freqs, Wq, bq, Wk, bk, Wv, bv, Wo, bo)
        nc = _get_kernel_fast(qk_bias)
        res = run_bass_kernel_spmd(nc, in_maps, list(range(NCORES)))
        acc = np.zeros((R, D), dtype=np.float32)
        for c in range(NCORES):
            acc += res.results[c]["out"].astype(np.float32)
    else:
        in_maps, qk_bias = _host_prep(x, mask, bias, rope_freqs, Wq, bq, Wk, bk, Wv, bv,
                                      Wo, bo, causal)
        nc = _get_kernel(causal, qk_bias)
        res = run_bass_kernel_spmd(nc, in_maps, list(range(NCORES)))
        acc = np.zeros((R, D), dtype=np.float32)
        for c in range(NCORES):
            acc += res.results[c]["out"]
    acc += bo.astype(np.float32)[None, :]
    if np.any(bv):
        acc += (bv.astype(np.float32) @ Wo.T.astype(np.float32))[None, :]
    return acc.reshape(B, S, D).astype(np.float32)

